# revision 24
# baseline (speedup 1.0000x reference)
"""Sharded DenseGNN Bass kernel for 8 TRN2 NeuronCores — v3.

v3 changes vs v2:
  - Layer 0 is fully replicated: the host packs (dinv*x)^T into 8-tile
    blocks and every core computes the FULL layer-0 gather tables locally
    (blocked matmuls against a block-diagonal W0'), writing f32 tables
    directly.  No AllGather and no fp8 cast for layer 0, and the layer-0
    gathers start ~as soon as the first table chunk is written.
  - dinv comes from the host (slab layout) instead of being recomputed
    on-device from pad counts; int1/int2 are no longer device inputs.
  - DRAM gather tables ping-pong by layer parity (L0,L2 -> set 0, L1 ->
    set 1) so a layer's cast never has a WAR hazard against the previous
    layer's in-flight gathers.
  - Pool-sequencer-aware emission: the Pool engine dispatches in order and
    parks on unmet semaphores, so the collective for the next layer is
    emitted a few calls AFTER its data deps are produced, and the fp8->f32
    casts ride SWDGE queue 1 (gathers on queue 0) emitted at points where
    their AG input is already (nearly) complete.
  - Encoder tail is software-pipelined in two phases with dedicated pools.
"""
import sys
import types

sys.path.insert(0, "/opt/trn_rl_repo")
if "antenv.axon_hooks" not in sys.modules:
    try:
        import antenv  # noqa: F401
        _m = types.ModuleType("antenv.axon_hooks")
        _m.get_axon_ntff_profile_hook = lambda: None
        sys.modules["antenv.axon_hooks"] = _m
    except ImportError:
        pass

import numpy as np

from concourse import bacc, bass, mybir, tile
from concourse.masks import make_identity

F32 = mybir.dt.float32
BF16 = mybir.dt.bfloat16
I32 = mybir.dt.int32
I16 = mybir.dt.int16
NG = 8
BN_EPS = 1e-5


class Config:
    def __init__(self, n_nodes, n_cores=8, max_idxs_per_call=12288, c2_t0=17):
        self.N = n_nodes
        self.C = n_cores
        self.NPC = n_nodes // n_cores
        self.TILE = 128
        self.TPC = (self.NPC + 127) // 128          # 49
        self.SPC = self.TPC * 128                   # 6272
        # chunk tile ranges (inclusive start, exclusive end)
        self.C1_T0, self.C1_T1 = 0, 32              # chunk1 = tiles 0..31
        self.C2_T0, self.C2_T1 = c2_t0, self.TPC    # chunk2 = tiles c2_t0..48
        self.C1_TILES = self.C1_T1 - self.C1_T0     # 32
        self.C2_TILES = self.C2_T1 - self.C2_T0     # 32
        self.ROWS1 = self.C * self.C1_TILES * 128   # 32768
        self.ROWS2 = self.C * self.C2_TILES * 128   # 32768
        assert self.ROWS1 <= 32768 and self.ROWS2 <= 32768
        # partial tile (dummy lanes) must sit inside the overlap zone
        self.PARTIAL_TILE = (self.C2_T0 + self.C1_T1) // 2
        assert self.C2_T0 <= self.PARTIAL_TILE < self.C1_T1
        self.LASTP = self.NPC - (self.TPC - 1) * 128   # real lanes in partial
        assert 0 < self.LASTP < 128
        # pad rows: core 0's partial-tile dummy lane (zeroed slab row)
        self.PAD1 = self.PARTIAL_TILE * 128 + self.LASTP          # chunk1 row
        self.PAD2 = (self.PARTIAL_TILE - self.C2_T0) * 128 + self.LASTP
        self.MAX_IDXS = max_idxs_per_call


CFG_FULL = Config(50000, max_idxs_per_call=3072)
SKIP_GATHER = False
SINGLE_PACKET = False    # True crashes the Q7 gather ucode
NQUEUES = 4              # gathers round-robin all SWDGE queues
FP8_AG = True            # fp8e4 AG tables halve collective bytes
GBUFS = 10
IL_DELAY = 2             # calls after tile-31 coverage before emitting AG1
CAST2_AT = 2             # calls into calls1 before emitting this layer's cast2
ENC_IL = True            # interleave encoder into layer-2 gather phase


def preprocess(cfg, edge_index, batch):
    """Build the SPMD-uniform schedule + per-core index data."""
    src_g = np.asarray(edge_index[0], dtype=np.int64)
    dst_g = np.asarray(edge_index[1], dtype=np.int64)
    N, C, NPC, SPC, TPC = cfg.N, cfg.C, cfg.NPC, cfg.SPC, cfg.TPC

    deg = np.bincount(dst_g, minlength=N).astype(np.int64) + 1  # incl self loop
    # pass 1: global degree sort into 49 bands of 1024 (zone structure);
    # the partial band (leftovers + dummies) is pinned to cfg.PARTIAL_TILE
    # inside the chunk-overlap (flex) zone.
    order_g = np.argsort(-deg, kind="stable")
    ntile_g = N // (C * 128)                  # 48 full bands
    band_tiles = [t for t in range(TPC) if t != cfg.PARTIAL_TILE]  # 48 slots
    gpos_tile = np.empty(N, dtype=np.int64)   # tile index of node
    pos = 0
    for k in range(ntile_g):
        gpos_tile[order_g[pos:pos + C * 128]] = band_tiles[k]
        pos += C * 128
    rem = order_g[pos:]                       # 848 leftovers -> partial tile
    gpos_tile[rem] = cfg.PARTIAL_TILE

    # per-node must counts vs the zone of each source's tile; zone membership
    # is invariant under within-zone regrouping, so these are exact.
    all_src = np.concatenate([src_g, np.arange(N)])
    all_dst = np.concatenate([dst_g, np.arange(N)])
    t_s0 = gpos_tile[all_src]
    e_cls = np.ones(all_src.shape, dtype=np.int8)
    e_cls[t_s0 < cfg.C2_T0] = 0
    e_cls[t_s0 >= cfg.C1_T1] = 2
    must1 = np.bincount(all_dst[e_cls == 0], minlength=N)
    flexc = np.bincount(all_dst[e_cls == 1], minlength=N)
    must2 = np.bincount(all_dst[e_cls == 2], minlength=N)
    cnt = must1 + flexc + must2

    # pass 2: within-zone 2-level regroup by (d1_0 desc, d2_0 desc) to
    # tighten per-tile maxima; partial tile stays pinned.
    d1_0 = np.clip((cnt + 1) // 2, must1, must1 + flexc)
    d2_0 = cnt - d1_0
    zones = ([t for t in range(0, cfg.C2_T0)],
             [t for t in range(cfg.C2_T0, cfg.C1_T1) if t != cfg.PARTIAL_TILE],
             [t for t in range(cfg.C1_T1, TPC)])
    for zone_tiles in zones:
        nodes = np.where(np.isin(gpos_tile, zone_tiles))[0]
        nt = len(zone_tiles)
        per = 1024
        assert nodes.size == nt * per
        order = np.lexsort((-d2_0[nodes], -d1_0[nodes]))
        nodes = nodes[order]
        n_super = int(np.ceil(np.sqrt(nt)))
        super_size = int(np.ceil(nt / n_super)) * per
        gi = 0
        for s0 in range(0, nodes.size, super_size):
            sb = nodes[s0:s0 + super_size]
            sb = sb[np.argsort(-d2_0[sb], kind="stable")]
            for g0 in range(0, sb.size, per):
                gpos_tile[sb[g0:g0 + per]] = zone_tiles[gi]
                gi += 1
        assert gi == nt

    # core/lane assignment within each tile
    gpos_core = np.empty(N, dtype=np.int64)
    gpos_lane = np.empty(N, dtype=np.int64)
    for t in range(TPC):
        nodes = np.where(gpos_tile == t)[0]
        lanes = cfg.LASTP if t == cfg.PARTIAL_TILE else 128
        assert nodes.size == C * lanes
        gpos_core[nodes] = np.arange(nodes.size) // lanes
        gpos_lane[nodes] = np.arange(nodes.size) % lanes

    # local slab position & perm layout: local = tile*128 + lane
    lpos = gpos_tile * 128 + gpos_lane
    perm_ok = np.zeros(C * SPC, dtype=np.int64) - 1
    perm_ok[gpos_core * SPC + lpos] = np.arange(N)
    perm = np.empty(N, dtype=np.int64)
    for c in range(C):
        slab = perm_ok[c * SPC:(c + 1) * SPC]
        perm[c * NPC:(c + 1) * NPC] = slab[slab >= 0]

    # chunk-table row of a source node
    t_s = gpos_tile
    row1 = gpos_core * (cfg.C1_TILES * 128) + t_s * 128 + gpos_lane
    row2 = gpos_core * (cfg.C2_TILES * 128) + (t_s - cfg.C2_T0) * 128 + gpos_lane

    # per-tile optimal split: a_t = argmin a + max(must2, cnt - a)
    d1_node = np.empty(N, dtype=np.int64)
    for t in range(TPC):
        idx = np.where(gpos_tile == t)[0]
        m1, fl, m2, c_ = must1[idx], flexc[idx], must2[idx], cnt[idx]
        amin, amax = int(m1.max()), int((m1 + fl).max())
        best = (1 << 60, amin, 0)
        for a in range(amin, amax + 1):
            b = int(np.maximum(m2, c_ - a).max())
            if a + b < best[0]:
                best = (a + b, a, b)
        _, a_t, b_t = best
        d1_node[idx] = np.maximum(m1, c_ - b_t)

    # per-core edge lists (dst-owned); self loops are real gather edges too
    owner = gpos_core[all_dst]
    d1_all = np.zeros((C, SPC), dtype=np.int64)
    d2_all = np.zeros((C, SPC), dtype=np.int64)
    per_core = []
    for c in range(C):
        m = owner == c
        es = all_src[m]
        ec = e_cls[m]
        dst_slot = lpos[all_dst[m]]
        order = np.lexsort((ec, dst_slot))
        es, ec, dst_slot = es[order], ec[order], dst_slot[order]
        counts = np.bincount(dst_slot, minlength=SPC)
        offs = np.concatenate([[0], np.cumsum(counts)])
        d1 = np.zeros(SPC, dtype=np.int64)
        nodes_c = np.where(gpos_core == c)[0]
        d1[lpos[nodes_c]] = d1_node[nodes_c]
        d2 = counts - d1
        assert (d2 >= 0).all()
        d1_all[c] = d1
        d2_all[c] = d2
        per_core.append(dict(es=es, offs=offs, d1=d1, d2=d2))

    # cross-core uniform per-tile slot widths
    D1 = np.maximum(d1_all.reshape(C, TPC, 128).max(axis=(0, 2)), 1)
    D2 = np.maximum(d2_all.reshape(C, TPC, 128).max(axis=(0, 2)), 1)
    S1, S2 = int(D1.sum()), int(D2.sum())
    off1 = np.concatenate([[0], np.cumsum(D1)]).astype(np.int64)
    off2 = np.concatenate([[0], np.cumsum(D2)]).astype(np.int64)

    int1 = np.full((C, 128, S1), cfg.PAD1, dtype=np.int32)
    int2 = np.full((C, 128, S2), cfg.PAD2, dtype=np.int32)
    for c in range(C):
        pc = per_core[c]
        es, offs, d1 = pc["es"], pc["offs"], pc["d1"]
        node_of_edge = np.repeat(np.arange(SPC), np.diff(offs))
        rank = np.arange(es.size) - offs[node_of_edge]
        is1 = rank < d1[node_of_edge]
        t_of = node_of_edge // 128
        p_of = node_of_edge % 128
        r1 = rank[is1]
        int1[c, p_of[is1], off1[t_of[is1]] + r1] = row1[es[is1]]
        r2 = rank[~is1] - d1[node_of_edge[~is1]]
        int2[c, p_of[~is1], off2[t_of[~is1]] + r2] = row2[es[~is1]]

    # gather-call schedule (uniform): 4 groups so dst tiles 0..31 are fully
    # finalized (both chunks) mid-phase -> next layer's chunk-1 slab + AG can
    # overlap the remaining gathers.
    calls = []
    idx_off = 0
    GROUPS = ((0, 0, cfg.C1_T1), (1, 0, cfg.C1_T1),
              (0, cfg.C1_T1, TPC), (1, cfg.C1_T1, TPC))
    for gi, (chunk, ts, te) in enumerate(GROUPS):
        D = D1 if chunk == 0 else D2
        t = ts
        while t < te:
            t0, nidx, chunks = t, 0, 0
            while t < te:
                d = int(D[t])
                if nidx + d * 128 > cfg.MAX_IDXS and t > t0:
                    break
                chunks += d
                nidx += d * 128
                t += 1
            calls.append(dict(chunk=chunk, group=gi, t0=t0, ntiles=t - t0,
                              idx_off=idx_off, nidx=nidx, chunks=chunks,
                              tile_D=[int(D[tt]) for tt in range(t0, t)]))
            idx_off += nidx
    total_idx = idx_off
    assert total_idx % 128 == 0
    T16 = total_idx // 16

    # per-core wrapped int16 idx lists
    idx16 = np.empty((C, 128, T16), dtype=np.int16)
    for c in range(C):
        flat = np.empty(total_idx, dtype=np.int16)
        for call in calls:
            tbl = int1[c] if call["chunk"] == 0 else int2[c]
            off = off1 if call["chunk"] == 0 else off2
            pos = call["idx_off"]
            for k, tt in enumerate(range(call["t0"], call["t0"] + call["ntiles"])):
                d = call["tile_D"][k]
                blk = tbl[:, off[tt]:off[tt] + d]      # [128, d]
                flat[pos:pos + d * 128] = blk.T.reshape(-1).astype(np.int16)
                pos += d * 128
            assert pos == call["idx_off"] + call["nidx"]
        wrapped = flat.reshape(-1, 16).T               # [16, T16]
        idx16[c] = np.tile(wrapped, (8, 1))

    sched = dict(cfg=cfg, D1=D1, D2=D2, S1=S1, S2=S2,
                 off1=off1, off2=off2, calls=calls, T16=T16,
                 perm=perm, lpos=lpos, gpos_core=gpos_core,
                 gpos_tile=gpos_tile, gpos_lane=gpos_lane, deg=deg)
    data = dict(idx16=idx16)
    return sched, data


def per_core_inputs(cfg, sched, data, inputs):
    """Build in_maps for run_bass_kernel_spmd."""
    x = np.asarray(inputs["x"], dtype=np.float32)
    batch = np.asarray(inputs["batch"], dtype=np.int64)
    lpos = sched["lpos"]
    gpos_core = sched["gpos_core"]
    gpos_tile = sched["gpos_tile"]
    gpos_lane = sched["gpos_lane"]
    deg = sched["deg"]
    C, NPC, SPC, TPC = cfg.C, cfg.NPC, cfg.SPC, cfg.TPC
    FIN = x.shape[1]

    # host: dinv and dinv-scaled x in slab layout (dummy lanes stay zero)
    dinv = 1.0 / np.sqrt(deg.astype(np.float64))
    xs = (x.astype(np.float64) * dinv[:, None]).astype(np.float32)
    Xslab = np.zeros((C, TPC, 128, FIN), dtype=np.float32)
    Xslab[gpos_core, gpos_tile, gpos_lane] = xs
    dinv_slab = np.zeros((C, 128, TPC), dtype=np.float32)
    dinv_slab[gpos_core, gpos_lane, gpos_tile] = dinv.astype(np.float32)

    # packed (dinv*x)^T blocks for replicated layer-0 table computation:
    # pass A covers chunk1 tiles (t in [0,32)), pass B chunk2 (t in [17,49)),
    # 8 tiles per 128-col group, 4 groups per core per pass.
    def pack_xt(trange):
        ngr = C * 4
        xt = np.zeros((3 * 8, ngr * 128), dtype=np.float32)
        for j in range(ngr):
            c, tg = j // 4, j % 4
            blk = Xslab[c, trange[0] + tg * 8: trange[0] + tg * 8 + 8]  # [8,128,3]
            xt[:, j * 128:(j + 1) * 128] = blk.transpose(0, 2, 1).reshape(24, 128)
        return np.ascontiguousarray(xt)

    xt1 = pack_xt((cfg.C1_T0, cfg.C1_T1))
    xt2 = pack_xt((cfg.C2_T0, cfg.C2_T1))

    common = {}
    for i in range(3):
        common[f"W{i}"] = np.ascontiguousarray(inputs[f"W{i}"], dtype=np.float32)
        for k in ("b", "g", "beta", "rm", "rv"):
            common[f"{k}{i}"] = np.ascontiguousarray(
                np.asarray(inputs[f"{k}{i}"], dtype=np.float32).reshape(-1, 1))
    common["enc_w0"] = np.ascontiguousarray(inputs["enc_w0"], dtype=np.float32)
    common["enc_w1"] = np.ascontiguousarray(inputs["enc_w1"], dtype=np.float32)
    common["dec_w0"] = np.ascontiguousarray(inputs["dec_w0"], dtype=np.float32)
    common["dec_w1"] = np.ascontiguousarray(inputs["dec_w1"], dtype=np.float32)
    for k in ("enc_b0", "enc_b1", "dec_b0", "dec_b1"):
        common[k] = np.ascontiguousarray(
            np.asarray(inputs[k], dtype=np.float32).reshape(-1, 1))
    common["xt1"] = xt1
    common["xt2"] = xt2

    in_maps = []
    for c in range(C):
        bp = np.full((SPC,), NG, dtype=np.int32)
        nodes_c = np.where(gpos_core == c)[0]
        bp[lpos[nodes_c]] = batch[nodes_c]
        bsb = bp.reshape(TPC, 128).T.astype(np.int32)  # [128, TPC]
        m = dict(common)
        m["batchp"] = np.ascontiguousarray(bsb)
        m["dinvl"] = np.ascontiguousarray(dinv_slab[c])
        m["idx16"] = np.ascontiguousarray(data["idx16"][c])
        in_maps.append(m)
    return in_maps


def build(cfg, sched, fin=3, body_repeat=1):
    C, TPC, SPC = cfg.C, cfg.TPC, cfg.SPC
    T16 = sched["T16"]
    calls = sched["calls"]
    cgroups = [[cl for cl in calls if cl["group"] == gi] for gi in range(4)]
    AL = mybir.AluOpType
    ROWS1, ROWS2 = cfg.ROWS1, cfg.ROWS2
    NGR = C * 4                                   # 8-tile groups per pass

    nc = bacc.Bacc(None, target_bir_lowering=False, debug=False, num_devices=C,
                   num_swdge_queues=NQUEUES)

    # ---- kernel I/O ----
    xt1_d = nc.dram_tensor("xt1", [24, NGR * 128], F32, kind="ExternalInput")
    xt2_d = nc.dram_tensor("xt2", [24, NGR * 128], F32, kind="ExternalInput")
    dinvl_d = nc.dram_tensor("dinvl", [128, TPC], F32, kind="ExternalInput")
    batch_d = nc.dram_tensor("batchp", [128, TPC], I32, kind="ExternalInput")
    idx16_d = nc.dram_tensor("idx16", [128, T16], I16, kind="ExternalInput")
    lp = []
    cins = [fin, 64, 128]
    for i in range(3):
        lp.append({
            "W": nc.dram_tensor(f"W{i}", [cins[i], 64], F32, kind="ExternalInput"),
            "b": nc.dram_tensor(f"b{i}", [64, 1], F32, kind="ExternalInput"),
            "g": nc.dram_tensor(f"g{i}", [64, 1], F32, kind="ExternalInput"),
            "beta": nc.dram_tensor(f"beta{i}", [64, 1], F32, kind="ExternalInput"),
            "rm": nc.dram_tensor(f"rm{i}", [64, 1], F32, kind="ExternalInput"),
            "rv": nc.dram_tensor(f"rv{i}", [64, 1], F32, kind="ExternalInput"),
        })
    encw0_d = nc.dram_tensor("enc_w0", [256, 128], F32, kind="ExternalInput")
    encb0_d = nc.dram_tensor("enc_b0", [128, 1], F32, kind="ExternalInput")
    encw1_d = nc.dram_tensor("enc_w1", [128, 64], F32, kind="ExternalInput")
    encb1_d = nc.dram_tensor("enc_b1", [64, 1], F32, kind="ExternalInput")
    decw0_d = nc.dram_tensor("dec_w0", [64, 32], F32, kind="ExternalInput")
    decb0_d = nc.dram_tensor("dec_b0", [32, 1], F32, kind="ExternalInput")
    decw1_d = nc.dram_tensor("dec_w1", [32, 1], F32, kind="ExternalInput")
    decb1_d = nc.dram_tensor("dec_b1", [1, 1], F32, kind="ExternalInput")
    out_d = nc.dram_tensor("out", [NG, 1], F32, kind="ExternalOutput")

    # ---- internal DRAM ----
    TDT = mybir.dt.float8e4 if FP8_AG else BF16
    slab1_d = nc.dram_tensor("slab1", [cfg.C1_TILES * 128, 64], TDT)
    slab2_d = nc.dram_tensor("slab2", [cfg.C2_TILES * 128, 64], TDT)
    # ping-pong table sets: set 0 for layers 0 and 2, set 1 for layer 1
    tbl_bf = [[nc.dram_tensor(f"tbl{ch}_bf{s}", [ROWS1 if ch == 0 else ROWS2, 64],
                              TDT, addr_space="Shared")
               for ch in (0, 1)] for s in (0, 1)]
    tbl_f = [[nc.dram_tensor(f"tbl{ch}_f{s}", [ROWS1 if ch == 0 else ROWS2, 64],
                             F32)
              for ch in (0, 1)] for s in (0, 1)]
    pool_in_d = nc.dram_tensor("pool_in", [NG, 65], F32)
    pool_ag_d = nc.dram_tensor("pool_ag", [C * NG, 65], F32, addr_space="Shared")

    from contextlib import ExitStack
    with tile.TileContext(nc) as tc, ExitStack() as es:
        const = es.enter_context(tc.tile_pool(name="const", bufs=1))
        work = es.enter_context(tc.tile_pool(name="work", bufs=3))
        gpool = es.enter_context(tc.tile_pool(name="gath", bufs=GBUFS))
        w512 = es.enter_context(tc.tile_pool(name="w512", bufs=3))
        ework = es.enter_context(tc.tile_pool(name="ework", bufs=3))
        pp = es.enter_context(tc.tile_pool(name="ps", bufs=3, space="PSUM"))
        eps = es.enter_context(tc.tile_pool(name="eps", bufs=2, space="PSUM"))
        ppacc = es.enter_context(tc.tile_pool(name="psacc", bufs=1, space="PSUM"))

        ident = const.tile([128, 128], F32, tag="ident")
        make_identity(nc, ident[:])
        ones_row = const.tile([1, 128], F32, tag="ones_row")
        nc.vector.memset(ones_row[:], 1.0)
        ones_col = const.tile([128, 1], F32, tag="ones_col")
        nc.vector.memset(ones_col[:], 1.0)
        iota8_i = const.tile([128, NG], I32, tag="iota8i")
        nc.gpsimd.iota(iota8_i[:], pattern=[[1, NG]], base=0, channel_multiplier=0)
        iota8 = const.tile([128, NG], F32, tag="iota8")
        nc.vector.tensor_copy(iota8[:], iota8_i[:])

        hfull = const.tile([128, TPC * 192], F32, tag="hfull")
        slabsb = const.tile([128, TPC * 64], TDT, tag="slabsb")
        dinv = const.tile([128, TPC], F32, tag="dinv")
        nc.sync.dma_start(out=dinv[:], in_=dinvl_d[:])
        batchsb = const.tile([128, TPC], I32, tag="batchsb")
        nc.sync.dma_start(out=batchsb[:], in_=batch_d[:])
        idx16sb = const.tile([128, T16], I16, tag="idx16sb")
        nc.sync.dma_start(out=idx16sb[:], in_=idx16_d[:])
        xt1sb = const.tile([24, NGR * 128], F32, tag="xt1sb")
        nc.sync.dma_start(out=xt1sb[:], in_=xt1_d[:])
        xt2sb = const.tile([24, NGR * 128], F32, tag="xt2sb")
        nc.sync.dma_start(out=xt2sb[:], in_=xt2_d[:])

        # ---- fold BN into W' and per-feature bias; broadcast rows ----
        wps = []
        bbcs = []
        for i in range(3):
            cin = cins[i]
            g64 = work.tile([64, 1], F32, tag="p64")
            rv64 = work.tile([64, 1], F32, tag="p64b")
            s64 = const.tile([64, 1], F32, tag=f"s64_{i}")
            nc.sync.dma_start(out=rv64[:], in_=lp[i]["rv"][:])
            nc.vector.tensor_scalar_add(rv64[:], rv64[:], BN_EPS)
            nc.scalar.sqrt(rv64[:], rv64[:])
            nc.vector.reciprocal(rv64[:], rv64[:])
            nc.sync.dma_start(out=g64[:], in_=lp[i]["g"][:])
            nc.vector.tensor_mul(s64[:], g64[:], rv64[:])
            # bias'' = (b - rm) * s + beta
            b64 = work.tile([64, 1], F32, tag="p64")
            rm64 = work.tile([64, 1], F32, tag="p64b")
            bb64 = const.tile([64, 1], F32, tag=f"bb64_{i}")
            nc.sync.dma_start(out=b64[:], in_=lp[i]["b"][:])
            nc.sync.dma_start(out=rm64[:], in_=lp[i]["rm"][:])
            nc.vector.tensor_sub(bb64[:], b64[:], rm64[:])
            nc.vector.tensor_mul(bb64[:], bb64[:], s64[:])
            be64 = work.tile([64, 1], F32, tag="p64")
            nc.sync.dma_start(out=be64[:], in_=lp[i]["beta"][:])
            nc.vector.tensor_add(bb64[:], bb64[:], be64[:])
            # transpose [64,1] -> [1,64], broadcast to [128,64]
            srow_ps = pp.tile([1, 64], F32, tag="ps", space="PSUM")
            nc.tensor.transpose(out=srow_ps[:], in_=s64[:], identity=ident[:64, :64])
            srow = work.tile([1, 64], F32, tag="row64")
            nc.vector.tensor_copy(srow[:], srow_ps[:])
            sbc_ps = pp.tile([128, 64], F32, tag="ps", space="PSUM")
            nc.tensor.matmul(out=sbc_ps[:], lhsT=ones_row[:1, :], rhs=srow[:],
                             start=True, stop=True)
            sbc = work.tile([128, 64], F32, tag="sbc")
            nc.vector.tensor_copy(sbc[:], sbc_ps[:])
            brow_ps = pp.tile([1, 64], F32, tag="ps", space="PSUM")
            nc.tensor.transpose(out=brow_ps[:], in_=bb64[:], identity=ident[:64, :64])
            brow = work.tile([1, 64], F32, tag="row64")
            nc.vector.tensor_copy(brow[:], brow_ps[:])
            bbc_ps = pp.tile([128, 64], F32, tag="ps", space="PSUM")
            nc.tensor.matmul(out=bbc_ps[:], lhsT=ones_row[:1, :], rhs=brow[:],
                             start=True, stop=True)
            bbc = const.tile([128, 64], F32, tag=f"bbc_{i}")
            nc.vector.tensor_copy(bbc[:], bbc_ps[:])
            bbcs.append(bbc)
            # W' = W * s (per output feature)
            wraw = work.tile([cin, 64], F32, tag="wraw")
            nc.sync.dma_start(out=wraw[:], in_=lp[i]["W"][:])
            wp = const.tile([cin, 64], F32, tag=f"wp_{i}")
            nc.vector.tensor_mul(wp[:], wraw[:], sbc[:cin, :])
            wps.append(wp)

        # block-diagonal W0' for the 8-tile grouped layer-0 matmuls
        w0blk = const.tile([24, 512], F32, tag="w0blk")
        nc.vector.memset(w0blk[:], 0.0)
        for i in range(8):
            nc.sync.dma_start(out=w0blk[3 * i:3 * i + 3, 64 * i:64 * i + 64],
                              in_=wps[0][:, :])

        # encoder weights: fold duplicated-h0 rows -> effective [192, 128];
        # encb0 folded in as a 65th row of the b-part (ones-row trick)
        encw0_a = const.tile([128, 128], F32, tag="encw0_a")
        encw0_b = const.tile([65, 128], F32, tag="encw0_b")
        nc.sync.dma_start(out=encw0_a[0:64, :], in_=encw0_d[0:64, :])
        nc.sync.dma_start(out=encw0_a[64:128, :], in_=encw0_d[128:192, :])
        tmpw = work.tile([64, 128], F32, tag="tmpw")
        nc.sync.dma_start(out=tmpw[:], in_=encw0_d[64:128, :])
        nc.vector.tensor_add(encw0_a[:64, :], encw0_a[:64, :], tmpw[:, :])
        nc.sync.dma_start(out=encw0_b[0:64, :], in_=encw0_d[192:256, :])
        eb0r_ps = pp.tile([1, 128], F32, tag="ps", space="PSUM")
        eb0v = work.tile([128, 1], F32, tag="pbias")
        nc.sync.dma_start(out=eb0v[:], in_=encb0_d[:])
        nc.tensor.transpose(out=eb0r_ps[:], in_=eb0v[:], identity=ident[:])
        nc.vector.tensor_copy(encw0_b[64:65, :], eb0r_ps[:])
        encw1 = const.tile([128, 64], F32, tag="encw1")
        nc.sync.dma_start(out=encw1[:], in_=encw1_d[:])
        encb1c = const.tile([64, 1], F32, tag="encb1c")
        nc.sync.dma_start(out=encb1c[:], in_=encb1_d[:])
        decw0 = const.tile([64, 32], F32, tag="decw0")
        nc.sync.dma_start(out=decw0[:], in_=decw0_d[:])
        decw1 = const.tile([32, 1], F32, tag="decw1")
        nc.sync.dma_start(out=decw1[:], in_=decw1_d[:])

        def bcast_bias(d_param, flen, parts, tag):
            v = work.tile([flen, 1], F32, tag="pbias")
            nc.sync.dma_start(out=v[:], in_=d_param[:])
            r_ps = pp.tile([1, flen], F32, tag="ps", space="PSUM")
            nc.tensor.transpose(out=r_ps[:], in_=v[:], identity=ident[:flen, :flen])
            r = work.tile([1, flen], F32, tag="rowb")
            nc.vector.tensor_copy(r[:], r_ps[:])
            b_ps = pp.tile([parts, flen], F32, tag="ps", space="PSUM")
            nc.tensor.matmul(out=b_ps[:], lhsT=ones_row[:1, :parts], rhs=r[:],
                             start=True, stop=True)
            b = const.tile([parts, flen], F32, tag=tag)
            nc.vector.tensor_copy(b[:], b_ps[:])
            return b

        encb0 = bcast_bias(encb0_d, 128, 128, "encb0")
        encb1 = bcast_bias(encb1_d, 64, 128, "encb1")
        decb0 = bcast_bias(decb0_d, 32, NG, "decb0")
        decb1 = bcast_bias(decb1_d, 1, NG, "decb1")

        # ---- GCN layers ----
        in_off = {0: None, 1: 0, 2: 0}     # layer1 reads h0 (64), layer2 h0|h1
        wr_off = {0: 0, 1: 64, 2: 128}
        AG_GROUPS = [list(range(C))]
        TSET = [0, 1, 0]                   # table set per layer

        def layer0_tables():
            """Replicated layer-0 tables: every core computes all rows, f32."""
            for xtsb, tbl in ((xt1sb, tbl_f[0][0]), (xt2sb, tbl_f[0][1])):
                for j in range(NGR):
                    mm_ps = pp.tile([128, 512], F32, tag="ps", space="PSUM")
                    nc.tensor.matmul(out=mm_ps[:],
                                     lhsT=xtsb[:, j * 128:(j + 1) * 128],
                                     rhs=w0blk[:], start=True, stop=True)
                    strip = w512.tile([128, 512], F32, tag="l0s")
                    nc.scalar.activation(strip[:], mm_ps[:],
                                         mybir.ActivationFunctionType.Copy)
                    nc.sync.dma_start(
                        out=tbl[j * 1024:(j + 1) * 1024, :].rearrange(
                            "(t p) f -> p t f", p=128),
                        in_=strip[:].rearrange("p (t f) -> p t f", f=64))

        def slab_tiles(li, t0, t1):
            """Compute slab tiles [t0, t1) for layer li and DMA them out to
            the chunk slab tensors (overlap tiles go to both)."""
            cin = cins[li]
            for t in range(t0, t1):
                o = t * 192 + in_off[li]
                hin = hfull[:, o:o + cin]
                tp_ps = pp.tile([cin, 128], F32, tag="ps", space="PSUM")
                nc.tensor.transpose(out=tp_ps[:], in_=hin, identity=ident[:])
                hT = work.tile([cin, 128], F32, tag="hT")
                nc.scalar.activation(hT[:], tp_ps[:],
                                     mybir.ActivationFunctionType.Copy)
                mm_ps = pp.tile([128, 64], F32, tag="ps", space="PSUM")
                nc.tensor.matmul(out=mm_ps[:], lhsT=hT[:], rhs=wps[li][:],
                                 start=True, stop=True)
                nc.vector.tensor_scalar_mul(
                    slabsb[:, t * 64:(t + 1) * 64], mm_ps[:], dinv[:, t:t + 1])
            # slab rows -> chunk slab DRAM tensors
            a0, a1 = max(t0, cfg.C1_T0), min(t1, cfg.C1_T1)
            if a0 < a1:
                nc.sync.dma_start(
                    out=slab1_d[(a0 - cfg.C1_T0) * 128:(a1 - cfg.C1_T0) * 128,
                                :].rearrange("(t p) f -> p t f", p=128),
                    in_=slabsb[:, a0 * 64:a1 * 64].rearrange(
                        "p (t f) -> p t f", f=64))
            b0, b1 = max(t0, cfg.C2_T0), min(t1, cfg.C2_T1)
            if b0 < b1:
                nc.sync.dma_start(
                    out=slab2_d[(b0 - cfg.C2_T0) * 128:(b1 - cfg.C2_T0) * 128,
                                :].rearrange("(t p) f -> p t f", p=128),
                    in_=slabsb[:, b0 * 64:b1 * 64].rearrange(
                        "p (t f) -> p t f", f=64))

        def allgather_chunk(which, tset):
            slab = slab1_d if which == 0 else slab2_d
            nc.gpsimd.collective_compute(
                "AllGather", AL.bypass, replica_groups=AG_GROUPS,
                ins=[slab[:]], outs=[tbl_bf[tset][which][:]])

        def cast_chunk(which, tset):
            # plain Pool DMAs ride SWDGE q0; the gathers ride q1, so a cast
            # parked on its AllGather input never head-blocks gather transfers
            nc.gpsimd.dma_start(out=tbl_f[tset][which][:],
                                in_=tbl_bf[tset][which][:])

        def gather_call(call, in_view, reduce_tile):
            """Issue one gather DMA, then per-tile segment reduces."""
            if not hasattr(gather_call, "q"):
                gather_call.q = 0
            chunks, nidx = call["chunks"], call["nidx"]
            buf = gpool.tile([128, chunks, 64], F32, tag="gbuf")
            c0 = call["idx_off"] // 16
            if not SKIP_GATHER:
                nc.gpsimd.dma_gather(
                    out_ap=buf[:], in_ap=in_view,
                    idxs_ap=idx16sb[:, c0:c0 + nidx // 16],
                    num_idxs=nidx, num_idxs_reg=nidx,
                    elem_size=64, queue_num=gather_call.q % NQUEUES,
                    single_packet=SINGLE_PACKET)
                gather_call.q += 1
            else:
                nc.gpsimd.memset(buf[:], 0.5)  # timing bisection stand-in
            ci = 0
            for k, tt in enumerate(range(call["t0"], call["t0"] + call["ntiles"])):
                d = call["tile_D"][k]
                seg = buf[:, ci:ci + d, :].rearrange("p d f -> p f d")
                ci += d
                reduce_tile(tt, seg)

        # ---- encoder phases (software-pipelined over tiles) ----
        # biases ride the matmuls (ones-row / per-partition Act bias), so the
        # per-tile chain is PE+Act only; DVE just stamps the count column.
        def enc_phase1(t):
            h2 = hfull[:, t * 192:(t + 1) * 192]
            e1_ps = eps.tile([128, 128], F32, tag="emm", space="PSUM")
            tp_ps = eps.tile([128, 128], F32, tag="etp", space="PSUM")
            nc.tensor.transpose(out=tp_ps[:], in_=h2[:, 0:128], identity=ident[:])
            hT = ework.tile([128, 128], F32, tag="ehT")
            nc.scalar.activation(hT[:], tp_ps[:], mybir.ActivationFunctionType.Copy)
            nc.tensor.matmul(out=e1_ps[:], lhsT=hT[:], rhs=encw0_a[:],
                             start=True, stop=False, skip_group_check=True)
            tp2_ps = eps.tile([64, 128], F32, tag="etp", space="PSUM")
            nc.tensor.transpose(out=tp2_ps[:], in_=h2[:, 128:192],
                                identity=ident[:])
            hTb = ework.tile([65, 128], F32, tag="ehTb")
            nc.scalar.activation(hTb[0:64, :], tp2_ps[:],
                                 mybir.ActivationFunctionType.Copy)
            nc.vector.tensor_copy(hTb[64:65, :], ones_row[:])
            nc.tensor.matmul(out=e1_ps[:], lhsT=hTb[:], rhs=encw0_b[:],
                             start=False, stop=True, skip_group_check=True)
            e1 = ework.tile([128, 128], F32, tag="e1")
            nc.scalar.activation(e1[:], e1_ps[:],
                                 mybir.ActivationFunctionType.Relu)
            return e1

        def enc_phase2(t, e1, pool_ps):
            tp3_ps = eps.tile([128, 128], F32, tag="etp", space="PSUM")
            nc.tensor.transpose(out=tp3_ps[:], in_=e1[:], identity=ident[:])
            e1T = ework.tile([128, 128], F32, tag="e1T")
            nc.scalar.activation(e1T[:], tp3_ps[:],
                                 mybir.ActivationFunctionType.Copy)
            # e2^T so the enc_b1 bias is per-partition and fuses with the relu
            e2T_ps = eps.tile([64, 128], F32, tag="emm", space="PSUM")
            nc.tensor.matmul(out=e2T_ps[:], lhsT=encw1[:], rhs=e1T[:],
                             start=True, stop=True, skip_group_check=True)
            e2T = ework.tile([64, 128], F32, tag="e2T")
            nc.scalar.activation(e2T[:], e2T_ps[:],
                                 mybir.ActivationFunctionType.Relu,
                                 bias=encb1c[:])
            e2_ps = eps.tile([128, 64], F32, tag="etp", space="PSUM")
            nc.tensor.transpose(out=e2_ps[:], in_=e2T[:], identity=ident[:64, :64])
            e2 = ework.tile([128, 65], F32, tag="e2")
            nc.scalar.activation(e2[:, :64], e2_ps[:],
                                 mybir.ActivationFunctionType.Copy)
            nc.vector.tensor_copy(e2[:, 64:65], ones_col[:])
            nc.tensor.matmul(out=pool_ps[:, :65], lhsT=oh_all[:, t * NG:(t + 1) * NG],
                             rhs=e2[:], start=(t == 0), stop=(t == TPC - 1),
                             skip_group_check=True)

        # ---- main body (repeatable for timing harnesses) ----
        for _rep in range(body_repeat):
            layer0_tables()
            pool_ps = ppacc.tile([NG, 65], F32, tag="pool", space="PSUM")
            pending_cast2 = None
            # one-hot pooling masks for all tiles in one DVE op
            oh_all = work.tile([128, TPC * NG], F32, tag="oh_all")
            btf_all = work.tile([128, TPC], F32, tag="btf_all")
            nc.vector.tensor_copy(btf_all[:], batchsb[:])
            nc.vector.tensor_tensor(
                out=oh_all[:].rearrange("p (t g) -> p t g", g=NG),
                in0=btf_all[:].rearrange("p (t o) -> p t o", o=1).to_broadcast(
                    [128, TPC, NG]),
                in1=iota8[:].rearrange("(o p) g -> p o g", o=1).to_broadcast(
                    [128, TPC, NG]),
                op=AL.is_equal)
            enc_st = {"p1": 0, "p2": 0, "e1": {}}

            def enc_advance(upto):
                """Advance the 2-phase encoder pipeline through tile `upto`."""
                while enc_st["p2"] < upto:
                    if enc_st["p1"] < TPC and enc_st["p1"] - enc_st["p2"] < 2:
                        t = enc_st["p1"]
                        enc_st["e1"][t] = enc_phase1(t)
                        enc_st["p1"] += 1
                    else:
                        t = enc_st["p2"]
                        enc_phase2(t, enc_st["e1"].pop(t), pool_ps)
                        enc_st["p2"] += 1
            for li in range(3):
                tset = TSET[li]
                w0 = wr_off[li]

                def reduce1(tt, seg, w0=w0):
                    mslice = hfull[:, tt * 192 + w0:tt * 192 + w0 + 64]
                    nc.vector.tensor_reduce(
                        mslice, seg, axis=mybir.AxisListType.X, op=AL.add)

                def reduce2(tt, seg, li=li, w0=w0):
                    mslice = hfull[:, tt * 192 + w0:tt * 192 + w0 + 64]
                    red = work.tile([128, 64], F32, tag="redB")
                    nc.vector.tensor_reduce(
                        red[:], seg, axis=mybir.AxisListType.X, op=AL.add)
                    nc.vector.tensor_add(red[:], red[:], mslice)
                    nc.vector.tensor_scalar_mul(red[:], red[:], dinv[:, tt:tt + 1])
                    nc.vector.tensor_add(red[:], red[:], bbcs[li][:])
                    nc.scalar.activation(
                        mslice, red[:], mybir.ActivationFunctionType.Relu)

                # group 0: chunk1 partials for dst tiles 0..31
                for call in cgroups[0]:
                    gather_call(call, tbl_f[tset][0][:], reduce1)
                # this layer's cast2 right before its first chunk2 reads;
                # its AG2 input was launched at the previous layer boundary
                if pending_cast2 is not None:
                    cast_chunk(1, pending_cast2)
                    pending_cast2 = None
                # group 1: chunk2 finalize for dst tiles 0..31
                for call in cgroups[1]:
                    gather_call(call, tbl_f[tset][1][:], reduce2)
                    if ENC_IL and li == 2:
                        enc_advance(call["t0"] + call["ntiles"] - 1)
                # group 2: chunk1 partials for dst 32..48; next layer's
                # chunk-1 slab + AG emitted IL_DELAY calls in (its deps -- the
                # dst 0..31 epilogues -- are met by then, so the Pool
                # sequencer doesn't park mid-gather-phase)
                emitted_nextA = False
                for k, call in enumerate(cgroups[2]):
                    gather_call(call, tbl_f[tset][0][:], reduce1)
                    if not emitted_nextA and li + 1 < 3 and k + 1 >= IL_DELAY:
                        slab_tiles(li + 1, cfg.C1_T0, cfg.C1_T1)
                        allgather_chunk(0, TSET[li + 1])
                        emitted_nextA = True
                if li + 1 < 3 and not emitted_nextA:
                    slab_tiles(li + 1, cfg.C1_T0, cfg.C1_T1)
                    allgather_chunk(0, TSET[li + 1])
                # group 3: chunk2 finalize for dst 32..48
                for call in cgroups[3]:
                    gather_call(call, tbl_f[tset][1][:], reduce2)
                    if ENC_IL and li == 2:
                        enc_advance(call["t0"] + call["ntiles"] - 1)
                if li + 1 < 3:
                    slab_tiles(li + 1, cfg.C1_T1, cfg.TPC)
                    allgather_chunk(1, TSET[li + 1])
                    cast_chunk(0, TSET[li + 1])
                    pending_cast2 = TSET[li + 1]

            # ---- encoder tail: drain whatever the interleave didn't cover ----
            enc_advance(TPC)
            poolsb = work.tile([NG, 65], F32, tag="poolsb")
            nc.vector.tensor_copy(poolsb[:], pool_ps[:])
            # ---- cross-core pool reduction: AllGather + local sum ----
            poolg = work.tile([NG, 65], F32, tag="poolg")
            nc.sync.dma_start(out=pool_in_d[:], in_=poolsb[:])
            nc.gpsimd.collective_compute(
                "AllGather", AL.bypass, replica_groups=AG_GROUPS,
                ins=[pool_in_d[:]], outs=[pool_ag_d[:]])
            pall = work.tile([NG, C, 65], F32, tag="pall")
            nc.sync.dma_start(
                out=pall[:], in_=pool_ag_d[:].rearrange("(c g) j -> g c j", g=NG))
            nc.vector.tensor_copy(poolg[:], pall[:, 0, :])
            for cc in range(1, C):
                nc.vector.tensor_add(poolg[:], poolg[:], pall[:, cc, :])
            # gfeat = pool / max(counts, 1)
            cnt = work.tile([NG, 1], F32, tag="cnt")
            nc.vector.tensor_scalar_max(cnt[:], poolg[:, 64:65], 1.0)
            nc.vector.reciprocal(cnt[:], cnt[:])
            gfeat = work.tile([NG, 64], F32, tag="gfeat")
            nc.vector.tensor_scalar_mul(gfeat[:], poolg[:, :64], cnt[:])
            # ---- decoder ----
            gfT_ps = pp.tile([64, NG], F32, tag="ps", space="PSUM")
            nc.tensor.transpose(out=gfT_ps[:], in_=gfeat[:], identity=ident[:NG, :NG])
            gfT = work.tile([64, NG], F32, tag="gfT")
            nc.vector.tensor_copy(gfT[:], gfT_ps[:])
            o1_ps = pp.tile([NG, 32], F32, tag="ps", space="PSUM")
            nc.tensor.matmul(out=o1_ps[:], lhsT=gfT[:], rhs=decw0[:],
                             start=True, stop=True, skip_group_check=True)
            o1 = work.tile([NG, 32], F32, tag="o1")
            nc.vector.tensor_add(o1[:], o1_ps[:], decb0[:])
            nc.scalar.activation(o1[:], o1[:], mybir.ActivationFunctionType.Relu)
            o1T_ps = pp.tile([32, NG], F32, tag="ps", space="PSUM")
            nc.tensor.transpose(out=o1T_ps[:], in_=o1[:], identity=ident[:NG, :NG])
            o1T = work.tile([32, NG], F32, tag="o1T")
            nc.vector.tensor_copy(o1T[:], o1T_ps[:])
            o2_ps = pp.tile([NG, 1], F32, tag="ps", space="PSUM")
            nc.tensor.matmul(out=o2_ps[:], lhsT=o1T[:], rhs=decw1[:],
                             start=True, stop=True, skip_group_check=True)
            o2 = work.tile([NG, 1], F32, tag="o2")
            nc.vector.tensor_add(o2[:], o2_ps[:], decb1[:])
            nc.sync.dma_start(out=out_d[:], in_=o2[:])

    nc.compile()
    return nc


_COMPILED = {}


def kernel(**inputs):
    """Full-input entry point: shards across 8 NeuronCores internally."""
    from concourse.bass_utils import run_bass_kernel_spmd

    cfg = CFG_FULL
    edge_index = np.asarray(inputs["edge_index"])
    batch = np.asarray(inputs["batch"])
    sched, data = preprocess(cfg, edge_index, batch)
    key = "full"
    if key not in _COMPILED:
        _COMPILED[key] = build(cfg, sched)
    nc = _COMPILED[key]
    in_maps = per_core_inputs(cfg, sched, data, inputs)
    res = run_bass_kernel_spmd(nc, in_maps, list(range(cfg.C)), trace=False)
    out = np.asarray(res.results[0]["out"])[:, 0].astype(np.float32)
    return out
